# revision 18
# baseline (speedup 1.0000x reference)
"""BayesianLinear Trainium2 kernel, 8-core SPMD (data-parallel over batch).

Per-core computation (4 samples each):
    w_b = weight_mean + noise_b * exp(0.5 * weight_logvar)   (B,O,I)
    out_b = x_b @ w_b^T + bias                               (B,L,O)

v7 design (per core), from the v1..v6 trace post-mortems:
  - The 16 shared DMA engines stream ~23.5 GB/s each on the f32 side, so
    the 32 MB of input reads are a fixed ~92 us of engine-pool time; the
    kernel is load-stream-bound and everything must hide under it.
  - SHORT dependency chains: noise is transposed RAW (DMA -> PE directly,
    no elementwise preprocessing), and the sampled weight is formed during
    the PSUM evacuation:  wT = nz^T * std^T + mean^T  (two DVE ops per
    o-block).  std^T and mean^T are transposed once into residents.  This
    removes a DVE hop ahead of every PE transpose - in v4-v6 that hop,
    plus per-hop queue latency, stretched sample 3's data-dependent tail
    from ~12 us of work to ~40 us.
  - x rows are loaded PERMUTED (row l = 4p + m, p = partition) so the
    bf16 output store is one DMA per sample with 8 KB contiguous
    per-partition packets.  Output is stored bf16 (tolerance 2e-2, lands
    ~4e-3) and widened to f32 on the host during the gather.
  - Loads carry NO scheduler ticks (virtual-time pins on DMA created
    artificial waits in v5/v6); compute is laddered with tile_wait_until
    in data-arrival order; stores are pinned late so they never steal
    engine slots from the load stream, draining during sample 3's tail.
  - Sample 3's GEMMs preload bias via a K=1 PE matmul and evacuate on the
    (tail-idle) scalar engine, keeping DVE off the tail critical path;
    its last noise chunk is split into single o-block loads so only
    ~6 us of work depends on the final bytes.  Sample 0 ramps with
    quarter-width GEMMs gated on single 1 MB chunks.
"""
import numpy as np

SAMPLES = 4           # batch samples per core
N_CORES = 8
B, L, I, O = 32, 512, 1024, 1024
KT = I // 128         # 8 k-tiles (contraction)
OT = O // 128         # 8 o-blocks
LT = L // 128         # 4 l-tile groups (interleaved rows 4p+m)

_cache = {}


def _split_multi_waits(nc, mybir):
    """This walrus build allows at most one sync-wait per instruction; move
    extra waits onto preceding single-wait NOPs on the same engine.  Safe
    because kernel semaphores are monotonic between resets, so waiting
    sequentially is equivalent to waiting on the conjunction."""
    for fn in nc.m.functions:
        for bb in fn.blocks:
            insts = bb.instructions
            changed = False
            new_list = []
            for inst in insts:
                si = inst.sync_info
                if si is not None and si.on_wait and len(si.on_wait) > 1:
                    waits = list(si.on_wait)
                    for j, w in enumerate(waits[:-1]):
                        nop = mybir.InstNoOp(name=f"{inst.name}-w{j}", ins=[], outs=[])
                        nop.engine = inst.engine
                        nop.sync_info = mybir.SyncInfo(on_wait=[w], on_update=[])
                        new_list.append(nop)
                    inst.sync_info = mybir.SyncInfo(
                        on_wait=[waits[-1]], on_update=list(si.on_update or []))
                    changed = True
                new_list.append(inst)
            if changed:
                bb.instructions = new_list


def build_nc(use_f32r=True):
    from contextlib import ExitStack
    from concourse import bass, mybir, tile, masks

    F32 = mybir.dt.float32
    BF16 = mybir.dt.bfloat16
    Exp = mybir.ActivationFunctionType.Exp
    Copy = mybir.ActivationFunctionType.Copy
    mult = mybir.AluOpType.mult
    add = mybir.AluOpType.add

    nc = bass.Bass()
    x_d = nc.declare_dram_parameter("x", [SAMPLES, L, I], F32, isOutput=False)
    nz_d = nc.declare_dram_parameter("noise", [SAMPLES, O, I], F32, isOutput=False)
    wm_d = nc.declare_dram_parameter("weight_mean", [O, I], F32, isOutput=False)
    wl_d = nc.declare_dram_parameter("weight_logvar", [O, I], F32, isOutput=False)
    b_d = nc.declare_dram_parameter("bias", [O], F32, isOutput=False)
    out_d = nc.declare_dram_parameter("out", [SAMPLES, L, O], BF16, isOutput=True)

    with tile.TileContext(nc) as tc, ExitStack() as ctx:
        resident = ctx.enter_context(tc.tile_pool(name="resident", bufs=1))
        biasp = ctx.enter_context(tc.tile_pool(name="biasp", bufs=1))
        lv_pool = ctx.enter_context(tc.tile_pool(name="lv", bufs=2))
        std_pool = ctx.enter_context(tc.tile_pool(name="stdp", bufs=2))
        mn_pool = ctx.enter_context(tc.tile_pool(name="mn", bufs=3))
        nz_pool = ctx.enter_context(tc.tile_pool(name="nz", bufs=2))
        xn_pool = ctx.enter_context(tc.tile_pool(name="xn", bufs=2))
        xT_pool = ctx.enter_context(tc.tile_pool(name="xT", bufs=2))
        wT_pool = ctx.enter_context(tc.tile_pool(name="wT", bufs=2))
        out_pool = ctx.enter_context(tc.tile_pool(name="outp", bufs=4))
        psum_mm = ctx.enter_context(tc.tile_pool(name="psum_mm", bufs=3, space="PSUM"))
        psum_nt = ctx.enter_context(tc.tile_pool(name="psum_nt", bufs=2, space="PSUM"))

        # ---------------- residents ----------------
        stdT = resident.tile([128, KT, O], BF16, tag="stdT")     # exp(.5 lv)^T
        meanT = resident.tile([128, KT, O], BF16, tag="meanT")   # mean^T
        ident_b = resident.tile([128, 128], BF16, tag="ident_b")
        ones_b = resident.tile([1, 128], BF16, tag="ones_b")
        bias_f = biasp.tile([1, O], F32, tag="bias_f")
        bias_b = resident.tile([1, O], BF16, tag="bias_b")
        bias_blk = resident.tile([128, O], F32, tag="bias_blk")  # bias bcast to rows

        lv_tiles, std_tiles, mn_tiles, nz_tiles, x_tiles = {}, {}, {}, {}, {}

        US = 0.001  # one microsecond of scheduler virtual time, in ms

        def at(t_us):
            return tc.tile_wait_until(t_us * US)

        # ---------------- DMA emitters (no ticks: stream order = priority)
        def emit_lv_load(j):
            lv_tiles[j] = lv_pool.tile([128, 2, I], F32, tag="lv", name=f"lv{j}")
            nc.sync.dma_start(
                lv_tiles[j][:],
                wl_d[256 * j:256 * (j + 1), :].rearrange("(q p) i -> p q i", p=128))

        def emit_mn_load(j):
            mn_tiles[j] = mn_pool.tile([128, 2, I], BF16, tag="mn", name=f"mn{j}")
            nc.gpsimd.dma_start(
                mn_tiles[j][:],
                wm_d[256 * j:256 * (j + 1), :].rearrange("(q p) i -> p q i", p=128))

        def emit_nz_load(b, clo, chi):
            if b not in nz_tiles:
                nz_tiles[b] = nz_pool.tile([128, OT, I], BF16, tag="nz",
                                           name=f"nz{b}")
            nc.gpsimd.dma_start(
                nz_tiles[b][:, 2 * clo:2 * chi, :],
                nz_d[b, 256 * clo:256 * chi, :].rearrange("(q p) i -> p q i", p=128))

        def emit_nz_load_ob(b, ob):
            nc.gpsimd.dma_start(
                nz_tiles[b][:, ob:ob + 1, :],
                nz_d[b, 128 * ob:128 * (ob + 1), :].rearrange("(q p) i -> p q i",
                                                              p=128))

        def emit_x_load(b, mlo, mhi):
            # permuted row mapping: row l = 4p + m -> 16 KB contiguous reads
            # per partition and DRAM-contiguous store packets later
            if b not in x_tiles:
                x_tiles[b] = xn_pool.tile([128, LT, I], BF16, tag="xn",
                                          name=f"xn{b}")
            nc.gpsimd.dma_start(
                x_tiles[b][:, mlo:mhi, :],
                x_d[b].rearrange("(p m) i -> p m i", p=128)[:, mlo:mhi, :])

        # ---------------- compute emitters ----------------
        def emit_exp(j):
            std_tiles[j] = std_pool.tile([128, 2, I], BF16, tag="std",
                                         name=f"std{j}")
            nc.scalar.activation(std_tiles[j][:], lv_tiles.pop(j)[:],
                                 Exp, bias=0.0, scale=0.5)

        def emit_stdT_group(j, q):
            """transpose std slab j half q -> stdT o-block 2j+q (PE + DVE)."""
            ob = 2 * j + q
            st = std_tiles[j] if q == 0 else std_tiles.pop(j)
            pt = psum_nt.tile([128, KT, 128], BF16, tag="pnt")
            for k in range(KT):
                nc.tensor.matmul(pt[:, k, :], st[:, q, 128 * k:128 * (k + 1)],
                                 ident_b[:], is_transpose=True, start=True, stop=True)
            nc.vector.tensor_copy(stdT[:, :, 128 * ob:128 * (ob + 1)], pt[:])

        def emit_mean_group(j, q):
            """transpose mean slab j half q -> meanT o-block 2j+q (PE + ACT)."""
            ob = 2 * j + q
            mn = mn_tiles[j] if q == 0 else mn_tiles.pop(j)
            pt = psum_nt.tile([128, KT, 128], BF16, tag="pnt")
            for k in range(KT):
                nc.tensor.matmul(pt[:, k, :], mn[:, q, 128 * k:128 * (k + 1)],
                                 ident_b[:], is_transpose=True, start=True, stop=True)
            nc.scalar.activation(meanT[:, :, 128 * ob:128 * (ob + 1)], pt[:], Copy)

        def emit_w_group(b, ob, wT, pop=False):
            """transpose RAW noise o-block; form w^T during the evac:
            wT[ob] = nz^T * std^T + mean^T   (PE + 2 DVE ops)."""
            nz = nz_tiles.pop(b) if pop else nz_tiles[b]
            pt = psum_nt.tile([128, KT, 128], BF16, tag="pnt")
            for k in range(KT):
                nc.tensor.matmul(pt[:, k, :], nz[:, ob, 128 * k:128 * (k + 1)],
                                 ident_b[:], is_transpose=True, start=True, stop=True)
            sl = slice(128 * ob, 128 * (ob + 1))
            nc.vector.tensor_tensor(wT[:, :, sl], pt[:], stdT[:, :, sl], mult)
            nc.vector.tensor_tensor(wT[:, :, sl], wT[:, :, sl], meanT[:, :, sl], add)

        def emit_xT_group(b, m, xT):
            xn = x_tiles.pop(b) if m == LT - 1 else x_tiles[b]
            pt = psum_nt.tile([128, KT, 128], BF16, tag="pnt")
            for k in range(KT):
                nc.tensor.matmul(pt[:, k, :], xn[:, m, 128 * k:128 * (k + 1)],
                                 ident_b[:], is_transpose=True, start=True, stop=True)
            nc.scalar.activation(xT[:, :, 128 * m:128 * (m + 1)], pt[:], Copy)

        def emit_gemm_full(m, wT, xT, ot):
            """all 1024 out-cols of tile m: shared stationary, 2 PSUM banks."""
            pm = psum_mm.tile([128, 2, 512], F32, tag="pmm")
            for k in range(KT):
                for n in range(2):
                    nc.tensor.matmul(pm[:, n, :], xT[:, k, 128 * m:128 * (m + 1)],
                                     wT[:, k, 512 * n:512 * (n + 1)],
                                     start=(k == 0), stop=(k == KT - 1))
            nc.vector.tensor_tensor(ot[:, m, :], pm[:].rearrange("p a b -> p (a b)"),
                                    bias_blk[:], add)

        def emit_gemm_half(m, n, wT, xT, ot, evac="dve"):
            pm = psum_mm.tile([128, 2, 512], F32, tag="pmm")
            sl = slice(512 * n, 512 * (n + 1))
            if evac == "scalar":
                # bias preloaded via K=1 PE matmul; scalar does the evac so
                # DVE stays off the tail critical path
                nc.tensor.matmul(pm[:, 0, :], ones_b[:], bias_b[:, sl],
                                 start=True, stop=False)
                for k in range(KT):
                    nc.tensor.matmul(pm[:, 0, :], xT[:, k, 128 * m:128 * (m + 1)],
                                     wT[:, k, sl],
                                     start=False, stop=(k == KT - 1))
                nc.scalar.activation(ot[:, m, sl], pm[:, 0, :], Copy)
            else:
                for k in range(KT):
                    nc.tensor.matmul(pm[:, 0, :], xT[:, k, 128 * m:128 * (m + 1)],
                                     wT[:, k, sl],
                                     start=(k == 0), stop=(k == KT - 1))
                nc.vector.tensor_tensor(ot[:, m, sl], pm[:, 0, :],
                                        bias_blk[:, sl], add)

        def emit_gemm_quarter(m, qo, wT, xT, ot, evac="dve"):
            pm = psum_mm.tile([128, 2, 512], F32, tag="pmm")
            sl = slice(256 * qo, 256 * (qo + 1))
            if evac == "scalar":
                nc.tensor.matmul(pm[:, 0, 0:256], ones_b[:], bias_b[:, sl],
                                 start=True, stop=False)
                for k in range(KT):
                    nc.tensor.matmul(pm[:, 0, 0:256],
                                     xT[:, k, 128 * m:128 * (m + 1)],
                                     wT[:, k, sl],
                                     start=False, stop=(k == KT - 1))
                nc.scalar.activation(ot[:, m, sl], pm[:, 0, 0:256], Copy)
            else:
                for k in range(KT):
                    nc.tensor.matmul(pm[:, 0, 0:256],
                                     xT[:, k, 128 * m:128 * (m + 1)],
                                     wT[:, k, sl],
                                     start=(k == 0), stop=(k == KT - 1))
                nc.vector.tensor_tensor(ot[:, m, sl], pm[:, 0, 0:256],
                                        bias_blk[:, sl], add)

        def emit_store(b, ot, mlo=0, mhi=LT):
            # row l = 4p + m: per-partition (m, o) block is DRAM-contiguous
            nc.scalar.dma_start(
                out_d[b].rearrange("(p m) o -> p m o", p=128)[:, mlo:mhi, :],
                ot[:, mlo:mhi, :])

        # ---------------- DMA stream (priority order, no ticks) -----------
        emit_lv_load(0)
        nc.sync.dma_start(bias_f[:], b_d[:].rearrange("(a o) -> a o", a=1))
        emit_lv_load(1)
        emit_x_load(0, 0, 1)
        emit_nz_load(0, 0, 1)
        emit_mn_load(0)
        masks.make_identity(nc, ident_b[:])
        emit_x_load(0, 1, 2)
        emit_nz_load(0, 1, 2)
        emit_mn_load(1)
        emit_nz_load(0, 2, 3)
        emit_mn_load(2)
        emit_x_load(0, 2, 3)
        emit_nz_load(0, 3, 4)
        emit_x_load(0, 3, 4)
        emit_mn_load(3)
        emit_lv_load(2)
        emit_lv_load(3)
        nc.vector.memset(ones_b[:], 1.0)

        # ---------------- setup compute ----------------
        with at(8):
            nc.vector.tensor_copy(bias_b[:], bias_f[:])
            for n in range(2):
                pb = psum_mm.tile([128, 2, 512], F32, tag="pmm", name=f"pb{n}")
                nc.tensor.matmul(pb[:, 0, :], ones_b[:],
                                 bias_b[:, 512 * n:512 * (n + 1)],
                                 start=True, stop=True)
                nc.scalar.activation(bias_blk[:, 512 * n:512 * (n + 1)],
                                     pb[:, 0, :], Copy)
            pw = psum_mm.tile([128, 2, 512], F32, tag="pmm", name="pw")
            for _ in range(4):
                nc.tensor.matmul(pw[:, 0, 0:128], ident_b[:], ident_b[:],
                                 start=True, stop=True)

        # ---------------- sample 0: fine-grained rampup ----------------
        def fresh(b):
            wT = wT_pool.tile([128, KT, O], BF16, tag="wT", name=f"wT{b}")
            xT = xT_pool.tile([128, KT, L], BF16, tag="xT", name=f"xT{b}")
            ot = out_pool.tile([128, LT, O], BF16, tag="out", name=f"ot{b}")
            return wT, xT, ot

        wT0, xT0, ot0 = fresh(0)
        ots = {0: ot0}
        with at(10):
            emit_exp(0)
        with at(12):
            emit_stdT_group(0, 0)
            emit_stdT_group(0, 1)
            emit_exp(1)
        with at(13):
            emit_mean_group(0, 0)
            emit_mean_group(0, 1)
            emit_xT_group(0, 0, xT0)
        with at(14):
            emit_w_group(0, 0, wT0)
            emit_w_group(0, 1, wT0)
            emit_exp(2)
        with at(15):
            emit_xT_group(0, 1, xT0)
            emit_stdT_group(1, 0)
            emit_stdT_group(1, 1)
        with at(16):
            emit_gemm_quarter(0, 0, wT0, xT0, ot0)
            emit_mean_group(1, 0)
            emit_mean_group(1, 1)
        with at(17):
            emit_w_group(0, 2, wT0)
            emit_w_group(0, 3, wT0)
            emit_exp(3)
        with at(18):
            emit_gemm_quarter(0, 1, wT0, xT0, ot0)
            emit_gemm_quarter(1, 0, wT0, xT0, ot0)
            emit_gemm_quarter(1, 1, wT0, xT0, ot0)
        with at(19):
            emit_stdT_group(2, 0)
            emit_stdT_group(2, 1)
            emit_mean_group(2, 0)
            emit_mean_group(2, 1)
        with at(20):
            emit_w_group(0, 4, wT0)
            emit_w_group(0, 5, wT0)
            emit_xT_group(0, 2, xT0)
        with at(21):
            emit_gemm_half(2, 0, wT0, xT0, ot0)
        with at(22):
            emit_stdT_group(3, 0)
            emit_stdT_group(3, 1)
            emit_mean_group(3, 0)
            emit_mean_group(3, 1)
        emit_nz_load(1, 0, 2)
        with at(23):
            emit_w_group(0, 6, wT0)
            emit_w_group(0, 7, wT0, pop=True)
            emit_xT_group(0, 3, xT0)
        with at(24):
            emit_gemm_half(3, 0, wT0, xT0, ot0)
        emit_x_load(1, 0, 4)
        with at(26):
            emit_gemm_half(0, 1, wT0, xT0, ot0)
            emit_gemm_half(1, 1, wT0, xT0, ot0)
        emit_nz_load(1, 2, 4)
        with at(28):
            emit_gemm_half(2, 1, wT0, xT0, ot0)
            emit_gemm_half(3, 1, wT0, xT0, ot0)

        # ---------------- samples 1, 2: full-width steady state ----------
        base = {1: 36, 2: 53, 3: 70}
        for b in (1, 2):
            wT, xT, ot = fresh(b)
            ots[b] = ot
            t = base[b]
            emit_nz_load(b + 1, 0, 2)
            with at(t):
                emit_w_group(b, 0, wT)
                emit_w_group(b, 1, wT)
            with at(t + 3):
                emit_w_group(b, 2, wT)
                emit_w_group(b, 3, wT)
            emit_x_load(b + 1, 0, 4)
            with at(t + 5):
                for m in range(LT):
                    emit_xT_group(b, m, xT)
            if b == 1:
                emit_nz_load(b + 1, 2, 4)
            else:
                emit_nz_load(b + 1, 2, 3)
            with at(t + 9):
                emit_w_group(b, 4, wT)
                emit_w_group(b, 5, wT)
            if b == 2:
                emit_nz_load_ob(b + 1, 6)
                emit_nz_load_ob(b + 1, 7)
            with at(t + 11):
                emit_w_group(b, 6, wT)
                emit_w_group(b, 7, wT, pop=True)
            with at(t + 13):
                for m in range(LT):
                    emit_gemm_full(m, wT, xT, ot)

        # ---------------- sample 3: minimal last-chunk dependency --------
        wT, xT, ot = fresh(3)
        ots[3] = ot
        t = base[3]
        with at(t):
            emit_w_group(3, 0, wT)
            emit_w_group(3, 1, wT)
        with at(t + 3):
            emit_w_group(3, 2, wT)
            emit_w_group(3, 3, wT)
        with at(t + 5):
            for m in range(LT):
                emit_xT_group(3, m, xT)
        with at(t + 7):
            for m in range(LT):
                emit_gemm_half(m, 0, wT, xT, ot, evac="scalar")
        with at(t + 11):
            emit_w_group(3, 4, wT)
            emit_w_group(3, 5, wT)
        with at(t + 12):
            for m in range(LT):
                emit_gemm_quarter(m, 2, wT, xT, ot, evac="scalar")
        with at(t + 15):
            emit_w_group(3, 6, wT)
        with at(t + 16):
            emit_w_group(3, 7, wT, pop=True)
        with at(t + 17):
            emit_gemm_quarter(0, 3, wT, xT, ot, evac="scalar")
            emit_gemm_quarter(1, 3, wT, xT, ot, evac="scalar")
        with at(t + 19):
            emit_store(3, ot, 0, 2)
            emit_gemm_quarter(2, 3, wT, xT, ot, evac="scalar")
            emit_gemm_quarter(3, 3, wT, xT, ot, evac="scalar")
        with at(t + 21):
            emit_store(3, ot, 2, 4)

        # deferred stores: behind the last noise load in the stream
        with at(92):
            emit_store(0, ots[0])
        with at(94):
            emit_store(1, ots[1])
        with at(96):
            emit_store(2, ots[2])

    _split_multi_waits(nc, mybir)
    return nc


def _get_nc(use_f32r=True):
    key = ("nc", use_f32r)
    if key not in _cache:
        _cache[key] = build_nc(use_f32r)
    return _cache[key]


def kernel(x, weight_mean, weight_logvar, bias, noise):
    from concourse import bass_utils

    x = np.ascontiguousarray(x, dtype=np.float32)
    noise = np.ascontiguousarray(noise, dtype=np.float32)
    weight_mean = np.ascontiguousarray(weight_mean, dtype=np.float32)
    weight_logvar = np.ascontiguousarray(weight_logvar, dtype=np.float32)
    bias = np.ascontiguousarray(bias, dtype=np.float32)

    nc = _get_nc()
    in_maps = []
    for c in range(N_CORES):
        sl = slice(SAMPLES * c, SAMPLES * (c + 1))
        in_maps.append({
            "x": x[sl], "noise": noise[sl],
            "weight_mean": weight_mean, "weight_logvar": weight_logvar,
            "bias": bias,
        })
    res = bass_utils.run_bass_kernel_spmd(nc, in_maps, list(range(N_CORES)))
    out = np.concatenate([np.asarray(res.results[c]["out"]).astype(np.float32)
                          for c in range(N_CORES)], axis=0)
    return out
